# revision 16
# baseline (speedup 1.0000x reference)
"""Multi-head causal self-attention on 8 trn2 NeuronCores, v3.

Problem: x[2,2048,1024], 16 heads x 64 dim, causal softmax attention,
QKV/O projections with biases.

Sharding: core c handles batch b=c//4, head group g=c%4 (heads 4g..4g+3).
Each core computes its 4 heads' attention plus the partial O-projection;
the host sums the 4 partials per batch and adds bo.

v3 design (vs v2):
- warm-up matmuls at kernel start: PE is idle ~4us waiting for the first
  DMAs; dummy matmuls on memset tiles fill that window AND ramp the PE
  HAM clock gate (1.2 -> 2.4GHz needs ~3.4us of sustained busy)
- projection order (k,q) x (dc0,dc1): the last proj tile is q-dc1 whose
  DVE bias-chunks don't gate attention qc0-pair0 (which needs only dc0);
  ACT k-copies split into 512-col chunks for finer overlap
- PV causal trimming: diagonal k-tiles only run PV on [lo:512] (the
  first k-tile of each accumulation is always full-width, so PSUM
  start/stop zero-region semantics stay valid); left-of-trapezoid
  memsets on p are gone
- denominator: vsb carries TWO tally columns (col 64 for even heads,
  col 65 for odd heads) so a pair's two denominator rows land on
  DIFFERENT psum partitions (64 and 65); one [2,128] selector matmul
  broadcasts both across 128 partitions, one reciprocal serves the pair
- qc3-pair1 tail: normalize per 128-col chunk, each chunk immediately
  feeding its o_stage_b matmul + add + store, instead of one monolithic
  normalize followed by 8 serialized O blocks
- xt row DMAs split across sync/gpsimd/scalar queues (serial on sync
  they gated early projection); output DMAs only on sync/gpsimd so ACT
  keeps the exp stream
"""
import os
import sys

if os.path.isdir("/opt/trn_rl_repo"):
    sys.path.insert(0, "/opt/trn_rl_repo")

import numpy as np
import ml_dtypes

import concourse.bass as bass  # noqa: F401
import concourse.tile as tile
from concourse import bacc
from concourse import mybir

F32 = mybir.dt.float32
F32R = mybir.dt.float32r
BF16 = mybir.dt.bfloat16
AF = mybir.ActivationFunctionType
ADD = mybir.AluOpType.add
MULT = mybir.AluOpType.mult

T = 2048          # sequence length
C = 1024          # model dim
HG = 4            # heads per core
HD = 64           # head dim
DG = HG * HD      # 256, projected dims per core
NF = C // 128     # 8 feature chunks
NT = T // 128     # 16 token tiles
NQ = T // 512     # 4 q-chunks
SCALE = 0.125     # 1/sqrt(64)
LAG = 8           # exp -> PV pipeline lag (in k-tiles)
NWARM = 8         # warm-up matmuls (512 cols each, ~0.4us cold)

BFNP = ml_dtypes.bfloat16


def build_kernel():
    nc = bacc.Bacc("TRN2")
    xT_d = nc.dram_tensor("xT", [C, T], BF16, kind="ExternalInput").ap()
    wq_d = nc.dram_tensor("wq", [128, NF * DG], BF16, kind="ExternalInput").ap()
    wk_d = nc.dram_tensor("wk", [128, NF * DG], BF16, kind="ExternalInput").ap()
    wv_d = nc.dram_tensor("wv", [128, NF * DG], BF16, kind="ExternalInput").ap()
    wo_d = nc.dram_tensor("wo", [2, 128, C], BF16, kind="ExternalInput").ap()
    bq_d = nc.dram_tensor("bq", [128, 2], F32, kind="ExternalInput").ap()
    bk_d = nc.dram_tensor("bk", [128, 2], F32, kind="ExternalInput").ap()
    bv_d = nc.dram_tensor("bv", [128, DG], F32, kind="ExternalInput").ap()
    out_d = nc.dram_tensor("out", [T, C], BF16, kind="ExternalOutput").ap()

    with tile.TileContext(nc) as tc:
        with tc.tile_pool(name="persist", bufs=1) as pp:
            qt = pp.tile([128, 2, T], BF16, name="qt")    # [d'128, pair, t]
            kt = pp.tile([128, 2, T], BF16, name="kt")
            # [V | even-tally | odd-tally]
            vsb = pp.tile([128, NT, HG, HD + 2], BF16, name="vsb")
            ctxT = [pp.tile([128, T], BF16, name=f"ctxT{p}") for p in range(2)]
            wo_sb = pp.tile([128, 2, C], BF16, name="wo_sb")
            bq_sb = pp.tile([128, 2], F32, name="bq_sb")
            bk_sb = pp.tile([128, 2], F32, name="bk_sb")
            bv_sb = pp.tile([128, DG], F32, name="bv_sb")
            mask2 = pp.tile([128, 2, 128], BF16, name="mask2")
            sel2 = pp.tile([128, 128], BF16, name="sel2")
            warm_w = pp.tile([128, 128], BF16, name="warm_w")
            warm_sb = pp.tile([128, 512], BF16, name="warm_sb")

            # all memsets on the DVE (idle until ~20us); gpsimd keeps its
            # queue free for DMA issues + the affine_selects (gpsimd-only
            # op). Warm-up matmuls depend only on the first two memsets.
            nc.vector.memset(warm_w[:], 0.001)
            nc.vector.memset(warm_sb[:], 0.001)
            # denominator tally columns: even heads tally into psum row
            # 64, odd heads into row 65 (so the pair's denominators land
            # on distinct partitions of their two psum tiles)
            nc.vector.memset(vsb[:, :, :, HD:HD + 2], 0.0)
            for h in range(HG):
                nc.vector.memset(vsb[:, :, h, HD + (h % 2):HD + (h % 2) + 1],
                                 1.0)
            nc.vector.memset(sel2[:], 1.0)
            nc.vector.memset(mask2[:], 1.0)
            with tc.tile_pool(name="xtp", bufs=1) as xtp, \
                 tc.tile_pool(name="wp", bufs=2) as wp:
                xt = xtp.tile([128, NF, T], BF16, name="xt")

                w_srcs = {"q": wq_d, "k": wk_d, "v": wv_d}
                w_tiles = {}

                def load_w(which, eng, split=False):
                    w_tiles[which] = wp.tile([128, NF, DG], BF16,
                                             name=f"w{which}", tag="w")
                    src_v = w_srcs[which].rearrange("p (f d) -> p f d", f=NF)
                    if split:
                        # f0 alone (64KB, gates the first matmul), then the
                        # rest as one contiguous 3.5KB-per-partition transfer
                        eng.dma_start(w_tiles[which][:, 0, :], src_v[:, 0, :])
                        eng.dma_start(w_tiles[which][:, 1:NF, :],
                                      src_v[:, 1:NF, :])
                    else:
                        eng.dma_start(w_tiles[which][:], src_v)

                def load_x(eng, f):
                    eng.dma_start(xt[:, f, :], xT_d[128 * f:128 * (f + 1), :])

                # queues ordered so each transfer lands just before its
                # first consumer: proj eats xt f-rows every ~1.2us from
                # ~10.5us on
                load_w("k", nc.scalar, split=True)   # wk f0 gates matmul #1
                # first chunk in 512-col pieces so the first matmul can
                # start as soon as 0.125MB lands
                for t4 in range(NQ):
                    nc.sync.dma_start(
                        xt[:, 0, 512 * t4:512 * (t4 + 1)],
                        xT_d[0:128, 512 * t4:512 * (t4 + 1)])
                # wq f0 gates the q-dc0 matmuls (interleaved with k-dc0
                # from the start); its tail rides behind xt f1
                w_tiles["q"] = wp.tile([128, NF, DG], BF16, name="wq",
                                       tag="w")
                wq_v = wq_d.rearrange("p (f d) -> p f d", f=NF)
                nc.gpsimd.dma_start(w_tiles["q"][:, 0, :], wq_v[:, 0, :])
                load_x(nc.gpsimd, 1)
                # tiny, and they gate the first kt/qt bias copies
                nc.sync.dma_start(bq_sb[:], bq_d)
                nc.sync.dma_start(bk_sb[:], bk_d)
                nc.gpsimd.dma_start(w_tiles["q"][:, 1:NF, :],
                                    wq_v[:, 1:NF, :])
                load_x(nc.scalar, 3)
                load_x(nc.sync, 2)
                load_x(nc.scalar, 5)
                load_x(nc.sync, 4)
                load_x(nc.gpsimd, 7)
                load_x(nc.scalar, 6)
                load_w("v", nc.gpsimd)
                nc.sync.dma_start(bv_sb[:], bv_d)
                for p in range(2):
                    nc.gpsimd.dma_start(wo_sb[:, p, :], wo_d[p])

                # gpsimd-only affine_selects, after its DMA issues (none
                # of these are needed before ~55us)
                # selector for the denominator broadcast matmul: psum row
                # 64 -> out partitions 0..63, row 65 -> partitions
                # 64..127. (partition starts must be 32-aligned, so rows
                # 64:66 are carved with two affine_selects)
                # keep iff col - 128p + 64 >= 0 (p rel. to partition 64)
                nc.gpsimd.affine_select(
                    out=sel2[64:66, :], in_=sel2[64:66, :],
                    compare_op=mybir.AluOpType.is_ge, fill=0.0,
                    base=64, pattern=[[1, 128]], channel_multiplier=-128)
                # keep iff -col + 63 + 128p >= 0
                nc.gpsimd.affine_select(
                    out=sel2[64:66, :], in_=sel2[64:66, :],
                    compare_op=mybir.AluOpType.is_ge, fill=0.0,
                    base=63, pattern=[[-1, 128]], channel_multiplier=128)
                # multiplicative triangle mask: 1 where col >= partition
                for j in range(2):
                    nc.gpsimd.affine_select(
                        out=mask2[:, j, :],
                        in_=mask2[:, j, :],
                        compare_op=mybir.AluOpType.is_ge,
                        fill=0.0,
                        base=0,
                        pattern=[[1, 128]],
                        channel_multiplier=-1,
                    )

                # ---- PE warm-up: fill the DMA wait, ramp the clock ----
                with tc.tile_pool(name="wrm", bufs=1, space="PSUM") as wmp:
                    warm_ps = wmp.tile([128, 512], F32, name="warm_ps")
                    for _ in range(NWARM):
                        nc.tensor.matmul(warm_ps[:], warm_w[:],
                                         warm_sb[:], start=True, stop=True)

                # ---- QK projections: psum [128, 2048] per (dst, dc) ----
                _sid_p, _ = nc.enter_named_scope("proj", False)
                with tc.tile_pool(name="pjp", bufs=2, space="PSUM") as pjp:
                    def copy_chunk(wkey, dst, ps, dc, b_sb, t4):
                        sl = slice(512 * t4, 512 * (t4 + 1))
                        if wkey == "k":
                            # ACT is idle during proj: bias + copy
                            nc.scalar.activation(
                                dst[:, dc, sl], ps[:, sl], AF.Identity,
                                bias=b_sb[:, dc:dc + 1])
                        else:
                            nc.vector.tensor_scalar_add(
                                dst[:, dc, sl], ps[:, sl], b_sb[:, dc:dc + 1])

                    # dc0: k and q interleaved f-outer. The start is
                    # aggregate-DMA-bound (~2us per 512KB xt row); a
                    # single proj tile eats a row per ~1us and stalls, two
                    # tiles together match the delivery rate so the PE
                    # stays continuously busy (HAM stays warm).
                    combos = (("k", kt, bk_sb), ("q", qt, bq_sb))
                    ps0 = {wkey: pjp.tile([128, T], F32, name=f"pj{wkey}0",
                                          tag="pj")
                           for wkey, _, _ in combos}
                    for f in range(NF - 1):
                        for wkey, dst, b_sb in combos:
                            for t4 in range(NQ):
                                nc.tensor.matmul(
                                    ps0[wkey][:, 512 * t4:512 * (t4 + 1)],
                                    w_tiles[wkey][:, f, 0:128],
                                    xt[:, f, 512 * t4:512 * (t4 + 1)],
                                    start=(f == 0), stop=False,
                                )
                    # last f-round chunk-by-chunk with inline copies so
                    # the psum banks retire while the tail chunks stream
                    for t4 in range(NQ):
                        for wkey, dst, b_sb in combos:
                            nc.tensor.matmul(
                                ps0[wkey][:, 512 * t4:512 * (t4 + 1)],
                                w_tiles[wkey][:, NF - 1, 0:128],
                                xt[:, NF - 1, 512 * t4:512 * (t4 + 1)],
                                start=False, stop=True,
                            )
                        for wkey, dst, b_sb in combos:
                            copy_chunk(wkey, dst, ps0[wkey], 0, b_sb, t4)
                    # dc1: x fully resident by now; t4-outer so each
                    # 512-chunk's psum retires right away (the attention
                    # pools reuse these banks)
                    ps1 = {wkey: pjp.tile([128, T], F32, name=f"pj{wkey}1",
                                          tag="pj")
                           for wkey, _, _ in combos}
                    for t4 in range(NQ):
                        for wkey, dst, b_sb in combos:
                            for f in range(NF):
                                nc.tensor.matmul(
                                    ps1[wkey][:, 512 * t4:512 * (t4 + 1)],
                                    w_tiles[wkey][:, f, 128:256],
                                    xt[:, f, 512 * t4:512 * (t4 + 1)],
                                    start=(f == 0), stop=(f == NF - 1),
                                )
                        for wkey, dst, b_sb in combos:
                            copy_chunk(wkey, dst, ps1[wkey], 1, b_sb, t4)
                nc.leave_named_scope("proj", _sid_p, False)

                # ---- phase B + V-proj/O-proj as PE filler ----
                wv_sb = w_tiles["v"]
                with tc.tile_pool(name="pp2", bufs=12) as pbuf, \
                     tc.tile_pool(name="opp", bufs=8) as opp, \
                     tc.tile_pool(name="outp", bufs=6) as outp, \
                     tc.tile_pool(name="dnp", bufs=4) as dnp, \
                     tc.tile_pool(name="sps", bufs=2, space="PSUM") as sps, \
                     tc.tile_pool(name="cps", bufs=2, space="PSUM") as cps, \
                     tc.tile_pool(name="vop", bufs=2, space="PSUM") as vop:

                    filler = []

                    def v_group(t):
                        def emit():
                            ps = vop.tile([128, DG], F32, name="vps", tag="vo")
                            for f in range(NF):
                                nc.tensor.matmul(
                                    ps[:],
                                    xt[:, f, 128 * t:128 * (t + 1)],
                                    wv_sb[:, f, :],
                                    start=(f == 0), stop=(f == NF - 1),
                                )
                            nc.vector.tensor_tensor(
                                vsb[:, t, :, 0:HD],
                                ps[:].rearrange("p (h d) -> p h d", h=HG),
                                bv_sb[:].rearrange("p (h d) -> p h d", h=HG),
                                ADD)
                        return emit

                    def o_stage_a(t0, c2, store):
                        def emit():
                            o_ps = vop.tile([128, 512], F32, name="opsA",
                                            tag="vo")
                            nc.tensor.matmul(
                                o_ps[:], ctxT[0][:, t0:t0 + 128],
                                wo_sb[:, 0, 512 * c2:512 * (c2 + 1)],
                                start=True, stop=True)
                            part = opp.tile([128, 512], BF16, name="opart",
                                            tag="op")
                            nc.vector.tensor_copy(part[:], o_ps[:])
                            store[(t0, c2)] = part
                        return emit

                    def o_stage_b(t0, c2, store):
                        o_ps = vop.tile([128, 512], F32, name="opsB",
                                        tag="vo")
                        nc.tensor.matmul(
                            o_ps[:], ctxT[1][:, t0:t0 + 128],
                            wo_sb[:, 1, 512 * c2:512 * (c2 + 1)],
                            start=True, stop=True)
                        o_sb = outp.tile([128, 512], BF16, name="osbB",
                                         tag="osb")
                        nc.vector.tensor_tensor(
                            o_sb[:], o_ps[:], store[(t0, c2)], ADD)
                        (nc.sync, nc.gpsimd)[c2].dma_start(
                            out_d[t0:t0 + 128, 512 * c2:512 * (c2 + 1)],
                            o_sb[:])

                    def o_group(t0, c2, alt=[0]):
                        def emit():
                            o_ps = vop.tile([128, 512], F32, name="ops",
                                            tag="vo")
                            for p in range(2):
                                nc.tensor.matmul(
                                    o_ps[:],
                                    ctxT[p][:, t0:t0 + 128],
                                    wo_sb[:, p, 512 * c2:512 * (c2 + 1)],
                                    start=(p == 0), stop=(p == 1),
                                )
                            o_sb = outp.tile([128, 512], BF16, name="osb",
                                             tag="osb")
                            nc.vector.tensor_copy(o_sb[:], o_ps[:])
                            nc.sync.dma_start(
                                out_d[t0:t0 + 128, 512 * c2:512 * (c2 + 1)],
                                o_sb[:])
                        return emit

                    for t in range(NT):
                        filler.append(v_group(t))

                    def pop_filler(n=1):
                        for _ in range(n):
                            if filler:
                                filler.pop(0)()

                    o_parts = {}

                    def make_norm(qc, pair, cps_t, granular):
                        """Denominator broadcast + normalize for a finished
                        pair. Deferred to just after the NEXT pair's first
                        S batch so the PE never idles on the pd casts."""
                        heads = (2 * pair, 2 * pair + 1)

                        def emit():
                            h0, h1 = heads
                            pd = dnp.tile([66, 512], BF16, name="pd",
                                          tag="dnb")
                            # odd head first as a 2-row aligned copy (its
                            # row 64 is a zero tally, overwritten next by
                            # the even head's single aligned row)
                            nc.vector.tensor_copy(pd[64:66, :],
                                                  cps_t[h1][64:66, :])
                            nc.vector.tensor_copy(pd[64:65, :],
                                                  cps_t[h0][64:65, :])
                            bc_ps = vop.tile([128, 512], F32, name="bc",
                                             tag="vo")
                            nc.tensor.matmul(
                                bc_ps[:],
                                sel2[64:66, :],
                                pd[64:66, :],
                                start=True, stop=True)
                            bcr = dnp.tile([128, 512], F32, name="bcr",
                                           tag="bcr")
                            nc.vector.reciprocal_approx_fast(
                                out=bcr[:], in_=bc_ps[:])
                            if not granular:
                                for j, h in enumerate(heads):
                                    nc.vector.tensor_tensor(
                                        ctxT[pair][64 * j:64 * j + 64,
                                                   512 * qc:512 * (qc + 1)],
                                        cps_t[h][0:HD, :],
                                        bcr[64 * j:64 * j + 64, :], MULT)
                            else:
                                # final chunk: normalize per 128-col piece,
                                # each immediately feeding its O block
                                for tt in range(4):
                                    csl = slice(128 * tt, 128 * (tt + 1))
                                    for j, h in enumerate(heads):
                                        nc.vector.tensor_tensor(
                                            ctxT[1][64 * j:64 * j + 64,
                                                    512 * qc + 128 * tt:
                                                    512 * qc + 128 * (tt + 1)],
                                            cps_t[h][0:HD, csl],
                                            bcr[64 * j:64 * j + 64, csl],
                                            MULT)
                                    for c2 in range(2):
                                        o_stage_b(512 * qc + 128 * tt, c2,
                                                  o_parts)
                        return emit

                    pending_norm = []

                    for qc in range(NQ):
                        _sid_a, _ = nc.enter_named_scope(f"attn{qc}", False)
                        nkt = 4 * qc + 4
                        for pair in range(2):
                            heads = (2 * pair, 2 * pair + 1)
                            cps_t = {h: cps.tile([66, 512], F32,
                                                 name=f"cps{h}", tag="cps")
                                     for h in heads}
                            pts = {}
                            los = {}
                            # 2-ki batches: 4 S matmuls, 2 exps, then 4 PV
                            # matmuls — longer same-shape PE runs
                            for kb in range(0, nkt + LAG, 2):
                                if kb < nkt:
                                    sts = {}
                                    for ki in (kb, kb + 1):
                                        r = ki - 4 * qc
                                        lo = 128 * r if r > 0 else 0
                                        sts[ki] = (lo, sps.tile(
                                            [128, 2, 512], F32,
                                            name="s_ps", tag="s"))
                                        for j in range(2):
                                            nc.tensor.matmul(
                                                sts[ki][1][:, j, lo:512],
                                                kt[64 * j:64 * j + 64, pair,
                                                   128 * ki:128 * (ki + 1)],
                                                qt[64 * j:64 * j + 64, pair,
                                                   512 * qc + lo:
                                                   512 * (qc + 1)],
                                                start=True, stop=True)
                                    for ki in (kb, kb + 1):
                                        lo, s_ps = sts[ki]
                                        r = ki - 4 * qc
                                        p_t = pbuf.tile([128, 2, 512], BF16,
                                                        name="p", tag="p")
                                        nc.scalar.activation(
                                            p_t[:, :, lo:512],
                                            s_ps[:, :, lo:512],
                                            AF.Exp, scale=SCALE)
                                        if r >= 0:
                                            # zero the above-diag triangle
                                            # (p is SBUF bf16 -> Pool-able)
                                            nc.gpsimd.tensor_tensor(
                                                p_t[:, :, lo:lo + 128],
                                                p_t[:, :, lo:lo + 128],
                                                mask2[:], MULT)
                                        pts[ki] = p_t
                                        los[ki] = lo
                                if kb == 2:
                                    # previous pair's normalize lands here,
                                    # behind 8 S matmuls of this pair — the
                                    # PE chews those while the DVE runs the
                                    # pd casts, so the broadcast matmul is
                                    # ready when the PE reaches it. All
                                    # filler pops come after this flush
                                    # (fillers read ctxT the norms write).
                                    while pending_norm:
                                        pending_norm.pop(0)()
                                if kb == 2:
                                    pop_filler(2)
                                elif kb > 2:
                                    pop_filler()
                                if kb >= LAG:
                                    for k in (kb - LAG, kb - LAG + 1):
                                        pk = pts.pop(k)
                                        lo = los.pop(k)
                                        for j, h in enumerate(heads):
                                            nc.tensor.matmul(
                                                cps_t[h][:, lo:512],
                                                vsb[:, k, h, :],
                                                pk[:, j, lo:512],
                                                start=(k == 0),
                                                stop=(k == nkt - 1),
                                            )
                            if qc == NQ - 1 and pair == 0:
                                for tt in range(4):
                                    for c2 in range(2):
                                        filler.append(o_stage_a(
                                            512 * qc + 128 * tt, c2, o_parts))
                            last = (qc == NQ - 1 and pair == 1)
                            pending_norm.append(
                                make_norm(qc, pair, cps_t, granular=last))
                            if last:
                                while pending_norm:
                                    pending_norm.pop(0)()

                        nc.leave_named_scope(f"attn{qc}", _sid_a, False)
                        if qc < NQ - 1:
                            for tt in range(4):
                                for c2 in range(2):
                                    filler.append(o_group(
                                        512 * qc + 128 * tt, c2))
                    while filler:
                        pop_filler()

    nc.compile()
    return nc


_NC_CACHE = None


def _get_nc():
    global _NC_CACHE
    if _NC_CACHE is None:
        _NC_CACHE = build_kernel()
    return _NC_CACHE


def make_in_maps(x, Wq, bq, Wk, bk, Wv, bv, Wo, bo):
    in_maps = []
    for c in range(8):
        b, g = c // 4, c % 4
        sl = slice(256 * g, 256 * (g + 1))
        bqg = np.ascontiguousarray(bq[sl].reshape(2, 128).T)
        bkg = np.ascontiguousarray(bk[sl].reshape(2, 128).T)
        bvg = np.ascontiguousarray(np.tile(bv[sl][None, :], (128, 1)))
        in_maps.append({
            "xT": np.ascontiguousarray(x[b].T).astype(BFNP),
            "wq": np.ascontiguousarray(
                Wq[:, sl].reshape(NF, 128, DG).transpose(1, 0, 2)
                .reshape(128, NF * DG)).astype(BFNP),
            "wk": np.ascontiguousarray(
                Wk[:, sl].reshape(NF, 128, DG).transpose(1, 0, 2)
                .reshape(128, NF * DG)).astype(BFNP),
            "wv": np.ascontiguousarray(
                Wv[:, sl].reshape(NF, 128, DG).transpose(1, 0, 2)
                .reshape(128, NF * DG)).astype(BFNP),
            "wo": np.ascontiguousarray(Wo[sl, :].reshape(2, 128, C)).astype(BFNP),
            "bq": bqg.astype(np.float32),
            "bk": bkg.astype(np.float32),
            "bv": bvg.astype(np.float32),
        })
    return in_maps


def combine_outputs(results, bo):
    out = np.empty((2, T, C), np.float32)
    for b in range(2):
        acc = results[4 * b]["out"].astype(np.float32).copy()
        for g in range(1, 4):
            acc += results[4 * b + g]["out"]
        out[b] = acc + bo[None, :]
    return out


def kernel(**inputs):
    from concourse.bass_utils import run_bass_kernel_spmd
    args = {k: np.asarray(v, np.float32) for k, v in inputs.items()}
    nc = _get_nc()
    in_maps = make_in_maps(
        args["x"], args["Wq"], args["bq"], args["Wk"], args["bk"],
        args["Wv"], args["bv"], args["Wo"], args["bo"])
    res = run_bass_kernel_spmd(nc, in_maps, core_ids=list(range(8)))
    return combine_outputs(res.results, args["bo"])


# revision 20
# speedup vs baseline: 1.0024x; 1.0024x over previous
"""Multi-head causal self-attention on 8 trn2 NeuronCores, v3.

Problem: x[2,2048,1024], 16 heads x 64 dim, causal softmax attention,
QKV/O projections with biases.

Sharding: core c handles batch b=c//4, head group g=c%4 (heads 4g..4g+3).
Each core computes its 4 heads' attention plus the partial O-projection;
the host sums the 4 partials per batch and adds bo.

v3 design (vs v2):
- warm-up matmuls at kernel start: PE is idle ~4us waiting for the first
  DMAs; dummy matmuls on memset tiles fill that window AND ramp the PE
  HAM clock gate (1.2 -> 2.4GHz needs ~3.4us of sustained busy)
- projection order (k,q) x (dc0,dc1): the last proj tile is q-dc1 whose
  DVE bias-chunks don't gate attention qc0-pair0 (which needs only dc0);
  ACT k-copies split into 512-col chunks for finer overlap
- PV causal trimming: diagonal k-tiles only run PV on [lo:512] (the
  first k-tile of each accumulation is always full-width, so PSUM
  start/stop zero-region semantics stay valid); left-of-trapezoid
  memsets on p are gone
- denominator: vsb carries TWO tally columns (col 64 for even heads,
  col 65 for odd heads) so a pair's two denominator rows land on
  DIFFERENT psum partitions (64 and 65); one [2,128] selector matmul
  broadcasts both across 128 partitions, one reciprocal serves the pair
- qc3-pair1 tail: normalize per 128-col chunk, each chunk immediately
  feeding its o_stage_b matmul + add + store, instead of one monolithic
  normalize followed by 8 serialized O blocks
- xt row DMAs split across sync/gpsimd/scalar queues (serial on sync
  they gated early projection); output DMAs only on sync/gpsimd so ACT
  keeps the exp stream
"""
import os
import sys

if os.path.isdir("/opt/trn_rl_repo"):
    sys.path.insert(0, "/opt/trn_rl_repo")

import numpy as np
import ml_dtypes

import concourse.bass as bass  # noqa: F401
import concourse.tile as tile
from concourse import bacc
from concourse import mybir

F32 = mybir.dt.float32
F32R = mybir.dt.float32r
BF16 = mybir.dt.bfloat16
AF = mybir.ActivationFunctionType
ADD = mybir.AluOpType.add
MULT = mybir.AluOpType.mult

T = 2048          # sequence length
C = 1024          # model dim
HG = 4            # heads per core
HD = 64           # head dim
DG = HG * HD      # 256, projected dims per core
NF = C // 128     # 8 feature chunks
NT = T // 128     # 16 token tiles
NQ = T // 512     # 4 q-chunks
SCALE = 0.125     # 1/sqrt(64)
LAG = 8           # exp -> PV pipeline lag (in k-tiles)
NWARM = 8         # warm-up matmuls (512 cols each, ~0.4us cold)

BFNP = ml_dtypes.bfloat16


def build_kernel():
    nc = bacc.Bacc("TRN2")
    xT_d = nc.dram_tensor("xT", [C, T], BF16, kind="ExternalInput").ap()
    wq_d = nc.dram_tensor("wq", [128, NF * DG], BF16, kind="ExternalInput").ap()
    wk_d = nc.dram_tensor("wk", [128, NF * DG], BF16, kind="ExternalInput").ap()
    wv_d = nc.dram_tensor("wv", [128, NF * DG], BF16, kind="ExternalInput").ap()
    wo_d = nc.dram_tensor("wo", [2, 128, C], BF16, kind="ExternalInput").ap()
    bq_d = nc.dram_tensor("bq", [128, 2], F32, kind="ExternalInput").ap()
    bk_d = nc.dram_tensor("bk", [128, 2], F32, kind="ExternalInput").ap()
    bv_d = nc.dram_tensor("bv", [128, DG], F32, kind="ExternalInput").ap()
    out_d = nc.dram_tensor("out", [T, C], BF16, kind="ExternalOutput").ap()

    with tile.TileContext(nc) as tc:
        with tc.tile_pool(name="persist", bufs=1) as pp:
            qt = pp.tile([128, 2, T], BF16, name="qt")    # [d'128, pair, t]
            kt = pp.tile([128, 2, T], BF16, name="kt")
            # [V | even-tally | odd-tally]
            vsb = pp.tile([128, NT, HG, HD + 2], BF16, name="vsb")
            ctxT = [pp.tile([128, T], BF16, name=f"ctxT{p}") for p in range(2)]
            wo_sb = pp.tile([128, 2, C], BF16, name="wo_sb")
            bq_sb = pp.tile([128, 2], F32, name="bq_sb")
            bk_sb = pp.tile([128, 2], F32, name="bk_sb")
            bv_sb = pp.tile([128, DG], F32, name="bv_sb")
            mask2 = pp.tile([128, 2, 128], BF16, name="mask2")
            sel2 = pp.tile([128, 128], BF16, name="sel2")
            warm_w = pp.tile([128, 128], BF16, name="warm_w")
            warm_sb = pp.tile([128, 512], BF16, name="warm_sb")

            # all memsets on the DVE (idle until ~20us); gpsimd keeps its
            # queue free for DMA issues + the affine_selects (gpsimd-only
            # op). Warm-up matmuls depend only on the first two memsets.
            # exactly 0.0: in-projection warmup matmuls ADD these into
            # live psum accumulations (bit-exact no-ops that keep the PE
            # busy while the DMA-bound xt rows trickle in)
            nc.vector.memset(warm_w[:], 0.0)
            nc.vector.memset(warm_sb[:], 0.0)
            # denominator tally columns: even heads tally into psum row
            # 64, odd heads into row 65 (so the pair's denominators land
            # on distinct partitions of their two psum tiles)
            nc.vector.memset(vsb[:, :, :, HD:HD + 2], 0.0)
            for h in range(HG):
                nc.vector.memset(vsb[:, :, h, HD + (h % 2):HD + (h % 2) + 1],
                                 1.0)
            nc.vector.memset(sel2[:], 1.0)
            nc.vector.memset(mask2[:], 1.0)
            with tc.tile_pool(name="xtp", bufs=1) as xtp, \
                 tc.tile_pool(name="wp", bufs=2) as wp:
                xt = xtp.tile([128, NF, T], BF16, name="xt")

                w_srcs = {"q": wq_d, "k": wk_d, "v": wv_d}
                w_tiles = {}

                def load_w(which, eng, split=False):
                    w_tiles[which] = wp.tile([128, NF, DG], BF16,
                                             name=f"w{which}", tag="w")
                    src_v = w_srcs[which].rearrange("p (f d) -> p f d", f=NF)
                    if split:
                        # f0 alone (64KB, gates the first matmul), then the
                        # rest as one contiguous 3.5KB-per-partition transfer
                        eng.dma_start(w_tiles[which][:, 0, :], src_v[:, 0, :])
                        eng.dma_start(w_tiles[which][:, 1:NF, :],
                                      src_v[:, 1:NF, :])
                    else:
                        eng.dma_start(w_tiles[which][:], src_v)

                def load_x(eng, f):
                    eng.dma_start(xt[:, f, :], xT_d[128 * f:128 * (f + 1), :])

                # queues ordered so each transfer lands just before its
                # first consumer: proj eats xt f-rows every ~1.2us from
                # ~10.5us on
                load_w("k", nc.scalar, split=True)   # wk f0 gates matmul #1
                # first chunk in 512-col pieces so the first matmul can
                # start as soon as 0.125MB lands
                for t4 in range(NQ):
                    nc.sync.dma_start(
                        xt[:, 0, 512 * t4:512 * (t4 + 1)],
                        xT_d[0:128, 512 * t4:512 * (t4 + 1)])
                # wq f0 gates the q-dc0 matmuls (interleaved with k-dc0
                # from the start); its tail rides behind xt f1
                w_tiles["q"] = wp.tile([128, NF, DG], BF16, name="wq",
                                       tag="w")
                wq_v = wq_d.rearrange("p (f d) -> p f d", f=NF)
                nc.gpsimd.dma_start(w_tiles["q"][:, 0, :], wq_v[:, 0, :])
                load_x(nc.gpsimd, 1)
                # tiny, and they gate the first kt/qt bias copies
                nc.sync.dma_start(bq_sb[:], bq_d)
                nc.sync.dma_start(bk_sb[:], bk_d)
                nc.gpsimd.dma_start(w_tiles["q"][:, 1:NF, :],
                                    wq_v[:, 1:NF, :])
                load_x(nc.scalar, 3)
                load_x(nc.sync, 2)
                load_x(nc.scalar, 5)
                load_x(nc.sync, 4)
                load_x(nc.gpsimd, 7)
                load_x(nc.scalar, 6)
                load_w("v", nc.gpsimd)
                nc.sync.dma_start(bv_sb[:], bv_d)
                for p in range(2):
                    nc.gpsimd.dma_start(wo_sb[:, p, :], wo_d[p])

                # gpsimd-only affine_selects, after its DMA issues (none
                # of these are needed before ~55us)
                # selector for the denominator broadcast matmul: psum row
                # 64 -> out partitions 0..63, row 65 -> partitions
                # 64..127. (partition starts must be 32-aligned, so rows
                # 64:66 are carved with two affine_selects)
                # keep iff col - 128p + 64 >= 0 (p rel. to partition 64)
                nc.gpsimd.affine_select(
                    out=sel2[64:66, :], in_=sel2[64:66, :],
                    compare_op=mybir.AluOpType.is_ge, fill=0.0,
                    base=64, pattern=[[1, 128]], channel_multiplier=-128)
                # keep iff -col + 63 + 128p >= 0
                nc.gpsimd.affine_select(
                    out=sel2[64:66, :], in_=sel2[64:66, :],
                    compare_op=mybir.AluOpType.is_ge, fill=0.0,
                    base=63, pattern=[[-1, 128]], channel_multiplier=128)
                # multiplicative triangle mask: 1 where col >= partition
                for j in range(2):
                    nc.gpsimd.affine_select(
                        out=mask2[:, j, :],
                        in_=mask2[:, j, :],
                        compare_op=mybir.AluOpType.is_ge,
                        fill=0.0,
                        base=0,
                        pattern=[[1, 128]],
                        channel_multiplier=-1,
                    )

                # ---- PE warm-up: fill the DMA wait, ramp the clock ----
                with tc.tile_pool(name="wrm", bufs=1, space="PSUM") as wmp:
                    warm_ps = wmp.tile([128, 512], F32, name="warm_ps")
                    for _ in range(NWARM):
                        nc.tensor.matmul(warm_ps[:], warm_w[:],
                                         warm_sb[:], start=True, stop=True)

                # ---- QK projections: psum [128, 2048] per (dst, dc) ----
                _sid_p, _ = nc.enter_named_scope("proj", False)
                with tc.tile_pool(name="pjp", bufs=2, space="PSUM") as pjp:
                    def copy_chunk(wkey, dst, ps, dc, b_sb, t4):
                        sl = slice(512 * t4, 512 * (t4 + 1))
                        if wkey == "k":
                            # ACT is idle during proj: bias + copy
                            nc.scalar.activation(
                                dst[:, dc, sl], ps[:, sl], AF.Identity,
                                bias=b_sb[:, dc:dc + 1])
                        else:
                            nc.vector.tensor_scalar_add(
                                dst[:, dc, sl], ps[:, sl], b_sb[:, dc:dc + 1])

                    # dc0: k and q interleaved f-outer. The start is
                    # aggregate-DMA-bound (~2us per 512KB xt row); a
                    # single proj tile eats a row per ~1us and stalls, two
                    # tiles together match the delivery rate so the PE
                    # stays continuously busy (HAM stays warm).
                    combos = (("k", kt, bk_sb), ("q", qt, bq_sb))
                    ps0 = {wkey: pjp.tile([128, T], F32, name=f"pj{wkey}0",
                                          tag="pj")
                           for wkey, _, _ in combos}
                    for f in range(NF - 1):
                        for wkey, dst, b_sb in combos:
                            for t4 in range(NQ):
                                nc.tensor.matmul(
                                    ps0[wkey][:, 512 * t4:512 * (t4 + 1)],
                                    w_tiles[wkey][:, f, 0:128],
                                    xt[:, f, 512 * t4:512 * (t4 + 1)],
                                    start=(f == 0), stop=False,
                                )
                        if f < NF - 2:
                            # xt rows arrive every ~3.5us (8 cores share
                            # HBM at start) but a round only takes ~2us:
                            # pad with +=0 matmuls so the PE never idles
                            # and the HAM clock stays warm
                            for w_i in range(3):
                                nc.tensor.matmul(
                                    ps0[combos[w_i % 2][0]][
                                        :, 512 * (w_i % NQ):
                                        512 * (w_i % NQ + 1)],
                                    warm_w[:],
                                    warm_sb[:],
                                    start=False, stop=False,
                                )
                    # last f-round chunk-by-chunk with inline copies so
                    # the psum banks retire while the tail chunks stream
                    for t4 in range(NQ):
                        for wkey, dst, b_sb in combos:
                            nc.tensor.matmul(
                                ps0[wkey][:, 512 * t4:512 * (t4 + 1)],
                                w_tiles[wkey][:, NF - 1, 0:128],
                                xt[:, NF - 1, 512 * t4:512 * (t4 + 1)],
                                start=False, stop=True,
                            )
                        for wkey, dst, b_sb in combos:
                            copy_chunk(wkey, dst, ps0[wkey], 0, b_sb, t4)
                    # dc1: x fully resident by now; t4-outer so each
                    # 512-chunk's psum retires right away (the attention
                    # pools reuse these banks)
                    ps1 = {wkey: pjp.tile([128, T], F32, name=f"pj{wkey}1",
                                          tag="pj")
                           for wkey, _, _ in combos}
                    for t4 in range(NQ):
                        for wkey, dst, b_sb in combos:
                            for f in range(NF):
                                nc.tensor.matmul(
                                    ps1[wkey][:, 512 * t4:512 * (t4 + 1)],
                                    w_tiles[wkey][:, f, 128:256],
                                    xt[:, f, 512 * t4:512 * (t4 + 1)],
                                    start=(f == 0), stop=(f == NF - 1),
                                )
                        for wkey, dst, b_sb in combos:
                            copy_chunk(wkey, dst, ps1[wkey], 1, b_sb, t4)
                nc.leave_named_scope("proj", _sid_p, False)

                # ---- phase B + V-proj/O-proj as PE filler ----
                wv_sb = w_tiles["v"]
                with tc.tile_pool(name="pp2", bufs=12) as pbuf, \
                     tc.tile_pool(name="opp", bufs=8) as opp, \
                     tc.tile_pool(name="outp", bufs=6) as outp, \
                     tc.tile_pool(name="dnp", bufs=4) as dnp, \
                     tc.tile_pool(name="sps", bufs=2, space="PSUM") as sps, \
                     tc.tile_pool(name="cps", bufs=2, space="PSUM") as cps, \
                     tc.tile_pool(name="vop", bufs=2, space="PSUM") as vop:

                    filler = []

                    def v_group(t):
                        def emit():
                            ps = vop.tile([128, DG], F32, name="vps", tag="vo")
                            for f in range(NF):
                                nc.tensor.matmul(
                                    ps[:],
                                    xt[:, f, 128 * t:128 * (t + 1)],
                                    wv_sb[:, f, :],
                                    start=(f == 0), stop=(f == NF - 1),
                                )
                            nc.vector.tensor_tensor(
                                vsb[:, t, :, 0:HD],
                                ps[:].rearrange("p (h d) -> p h d", h=HG),
                                bv_sb[:].rearrange("p (h d) -> p h d", h=HG),
                                ADD)
                        return emit

                    def o_stage_a(t0, c2, store):
                        def emit():
                            o_ps = vop.tile([128, 512], F32, name="opsA",
                                            tag="vo")
                            nc.tensor.matmul(
                                o_ps[:], ctxT[0][:, t0:t0 + 128],
                                wo_sb[:, 0, 512 * c2:512 * (c2 + 1)],
                                start=True, stop=True)
                            part = opp.tile([128, 512], BF16, name="opart",
                                            tag="op")
                            nc.vector.tensor_copy(part[:], o_ps[:])
                            store[(t0, c2)] = part
                        return emit

                    def o_stage_b(t0, c2, store):
                        o_ps = vop.tile([128, 512], F32, name="opsB",
                                        tag="vo")
                        nc.tensor.matmul(
                            o_ps[:], ctxT[1][:, t0:t0 + 128],
                            wo_sb[:, 1, 512 * c2:512 * (c2 + 1)],
                            start=True, stop=True)
                        o_sb = outp.tile([128, 512], BF16, name="osbB",
                                         tag="osb")
                        nc.vector.tensor_tensor(
                            o_sb[:], o_ps[:], store[(t0, c2)], ADD)
                        (nc.sync, nc.gpsimd)[c2].dma_start(
                            out_d[t0:t0 + 128, 512 * c2:512 * (c2 + 1)],
                            o_sb[:])

                    def o_group(t0, c2, alt=[0]):
                        def emit():
                            o_ps = vop.tile([128, 512], F32, name="ops",
                                            tag="vo")
                            for p in range(2):
                                nc.tensor.matmul(
                                    o_ps[:],
                                    ctxT[p][:, t0:t0 + 128],
                                    wo_sb[:, p, 512 * c2:512 * (c2 + 1)],
                                    start=(p == 0), stop=(p == 1),
                                )
                            o_sb = outp.tile([128, 512], BF16, name="osb",
                                             tag="osb")
                            nc.vector.tensor_copy(o_sb[:], o_ps[:])
                            nc.sync.dma_start(
                                out_d[t0:t0 + 128, 512 * c2:512 * (c2 + 1)],
                                o_sb[:])
                        return emit

                    for t in range(NT):
                        filler.append(v_group(t))

                    def pop_filler(n=1):
                        for _ in range(n):
                            if filler:
                                filler.pop(0)()

                    o_parts = {}

                    def make_norm(qc, pair, cps_t, granular):
                        """Denominator broadcast + normalize for a finished
                        pair. Deferred to just after the NEXT pair's first
                        S batch so the PE never idles on the pd casts."""
                        heads = (2 * pair, 2 * pair + 1)

                        def emit():
                            h0, h1 = heads
                            pd = dnp.tile([66, 512], BF16, name="pd",
                                          tag="dnb")
                            # odd head first as a 2-row aligned copy (its
                            # row 64 is a zero tally, overwritten next by
                            # the even head's single aligned row). At the
                            # tail the casts go to the (by then idle) ACT
                            # so the DVE chain stays short.
                            ceng = nc.scalar if granular else nc.vector
                            if granular:
                                ceng.activation(pd[64:66, :],
                                                cps_t[h1][64:66, :], AF.Copy)
                                ceng.activation(pd[64:65, :],
                                                cps_t[h0][64:65, :], AF.Copy)
                            else:
                                ceng.tensor_copy(pd[64:66, :],
                                                 cps_t[h1][64:66, :])
                                ceng.tensor_copy(pd[64:65, :],
                                                 cps_t[h0][64:65, :])
                            bc_ps = vop.tile([128, 512], F32, name="bc",
                                             tag="vo")
                            nc.tensor.matmul(
                                bc_ps[:],
                                sel2[64:66, :],
                                pd[64:66, :],
                                start=True, stop=True)
                            bcr = dnp.tile([128, 512], F32, name="bcr",
                                           tag="bcr")
                            nc.vector.reciprocal_approx_fast(
                                out=bcr[:], in_=bc_ps[:])
                            if not granular:
                                for j, h in enumerate(heads):
                                    nc.vector.tensor_tensor(
                                        ctxT[pair][64 * j:64 * j + 64,
                                                   512 * qc:512 * (qc + 1)],
                                        cps_t[h][0:HD, :],
                                        bcr[64 * j:64 * j + 64, :], MULT)
                            else:
                                # final chunk: normalize per 128-col piece,
                                # each immediately feeding its O block
                                for tt in range(4):
                                    csl = slice(128 * tt, 128 * (tt + 1))
                                    for j, h in enumerate(heads):
                                        nc.vector.tensor_tensor(
                                            ctxT[1][64 * j:64 * j + 64,
                                                    512 * qc + 128 * tt:
                                                    512 * qc + 128 * (tt + 1)],
                                            cps_t[h][0:HD, csl],
                                            bcr[64 * j:64 * j + 64, csl],
                                            MULT)
                                    for c2 in range(2):
                                        o_stage_b(512 * qc + 128 * tt, c2,
                                                  o_parts)
                        return emit

                    # (stamp, emit_fn): flushed one global step after they
                    # were appended, right behind that step's S batch
                    pending_norm = []
                    gstep = [0]

                    for qc in range(NQ):
                        _sid_a, _ = nc.enter_named_scope(f"attn{qc}", False)
                        nkt = 4 * qc + 4
                        # both pairs in one pipelined stream: the next
                        # pair's S batches run during the previous pair's
                        # PV drain, so the ACT exp stream never restarts
                        # cold and the PE has S work at every boundary
                        batches = [(p, kb) for p in (0, 1)
                                   for kb in range(0, nkt, 2)]
                        n_b = len(batches)
                        cps_all = {}
                        pts = {}
                        los = {}
                        for step in range(n_b + LAG // 2):
                            g = gstep[0]
                            if step < n_b:
                                pair, kb = batches[step]
                                if kb == 0:
                                    cps_all[pair] = {
                                        h: cps.tile([66, 512], F32,
                                                    name=f"cps{h}",
                                                    tag="cps")
                                        for h in (2 * pair, 2 * pair + 1)}
                                sts = {}
                                for ki in (kb, kb + 1):
                                    r = ki - 4 * qc
                                    lo = 128 * r if r > 0 else 0
                                    sts[ki] = (lo, sps.tile(
                                        [128, 2, 512], F32,
                                        name="s_ps", tag="s"))
                                    for j in range(2):
                                        nc.tensor.matmul(
                                            sts[ki][1][:, j, lo:512],
                                            kt[64 * j:64 * j + 64, pair,
                                               128 * ki:128 * (ki + 1)],
                                            qt[64 * j:64 * j + 64, pair,
                                               512 * qc + lo:
                                               512 * (qc + 1)],
                                            start=True, stop=True)
                                for ki in (kb, kb + 1):
                                    lo, s_ps = sts[ki]
                                    r = ki - 4 * qc
                                    p_t = pbuf.tile([128, 2, 512], BF16,
                                                    name="p", tag="p")
                                    nc.scalar.activation(
                                        p_t[:, :, lo:512],
                                        s_ps[:, :, lo:512],
                                        AF.Exp, scale=SCALE)
                                    if r >= 0:
                                        # zero the above-diag triangle
                                        # (p is SBUF bf16 -> Pool-able)
                                        nc.gpsimd.tensor_tensor(
                                            p_t[:, :, lo:lo + 128],
                                            p_t[:, :, lo:lo + 128],
                                            mask2[:], MULT)
                                    pts[(pair, ki)] = p_t
                                    los[(pair, ki)] = lo
                            # deferred normalizes land here: behind the S
                            # batch just emitted, ahead of this step's PV
                            # (whose psum banks they must release first)
                            while pending_norm and g > pending_norm[0][0]:
                                pending_norm.pop(0)[1]()
                            if step == 2:
                                pop_filler(2)
                            elif step > 2:
                                pop_filler()
                            bi = step - LAG // 2
                            if bi >= 0:
                                pair2, kb2 = batches[bi]
                                heads2 = (2 * pair2, 2 * pair2 + 1)
                                for k in (kb2, kb2 + 1):
                                    pk = pts.pop((pair2, k))
                                    lo = los.pop((pair2, k))
                                    for j, h in enumerate(heads2):
                                        nc.tensor.matmul(
                                            cps_all[pair2][h][:, lo:512],
                                            vsb[:, k, h, :],
                                            pk[:, j, lo:512],
                                            start=(k == 0),
                                            stop=(k == nkt - 1),
                                        )
                                if kb2 == nkt - 2:
                                    last = (qc == NQ - 1 and pair2 == 1)
                                    cps_p = cps_all[pair2]
                                    if qc == NQ - 1 and pair2 == 0:
                                        norm_fn = make_norm(qc, pair2, cps_p,
                                                            granular=False)

                                        def entry(norm_fn=norm_fn, qc=qc):
                                            norm_fn()
                                            # stage_a reads the ctxT the
                                            # norm just wrote — append only
                                            # now so no pop precedes it
                                            for tt in range(4):
                                                for c2 in range(2):
                                                    filler.append(o_stage_a(
                                                        512 * qc + 128 * tt,
                                                        c2, o_parts))
                                    else:
                                        entry = make_norm(qc, pair2, cps_p,
                                                          granular=last)
                                    if last:
                                        entry()
                                    else:
                                        pending_norm.append((g, entry))
                            gstep[0] += 1
                        nc.leave_named_scope(f"attn{qc}", _sid_a, False)
                        if qc < NQ - 1:
                            for tt in range(4):
                                for c2 in range(2):
                                    filler.append(o_group(
                                        512 * qc + 128 * tt, c2))
                    while pending_norm:
                        pending_norm.pop(0)[1]()
                    while filler:
                        pop_filler()

    nc.compile()
    return nc


_NC_CACHE = None


def _get_nc():
    global _NC_CACHE
    if _NC_CACHE is None:
        _NC_CACHE = build_kernel()
    return _NC_CACHE


def make_in_maps(x, Wq, bq, Wk, bk, Wv, bv, Wo, bo):
    in_maps = []
    for c in range(8):
        b, g = c // 4, c % 4
        sl = slice(256 * g, 256 * (g + 1))
        bqg = np.ascontiguousarray(bq[sl].reshape(2, 128).T)
        bkg = np.ascontiguousarray(bk[sl].reshape(2, 128).T)
        bvg = np.ascontiguousarray(np.tile(bv[sl][None, :], (128, 1)))
        in_maps.append({
            "xT": np.ascontiguousarray(x[b].T).astype(BFNP),
            "wq": np.ascontiguousarray(
                Wq[:, sl].reshape(NF, 128, DG).transpose(1, 0, 2)
                .reshape(128, NF * DG)).astype(BFNP),
            "wk": np.ascontiguousarray(
                Wk[:, sl].reshape(NF, 128, DG).transpose(1, 0, 2)
                .reshape(128, NF * DG)).astype(BFNP),
            "wv": np.ascontiguousarray(
                Wv[:, sl].reshape(NF, 128, DG).transpose(1, 0, 2)
                .reshape(128, NF * DG)).astype(BFNP),
            "wo": np.ascontiguousarray(Wo[sl, :].reshape(2, 128, C)).astype(BFNP),
            "bq": bqg.astype(np.float32),
            "bk": bkg.astype(np.float32),
            "bv": bvg.astype(np.float32),
        })
    return in_maps


def combine_outputs(results, bo):
    out = np.empty((2, T, C), np.float32)
    for b in range(2):
        acc = results[4 * b]["out"].astype(np.float32).copy()
        for g in range(1, 4):
            acc += results[4 * b + g]["out"]
        out[b] = acc + bo[None, :]
    return out


def kernel(**inputs):
    from concourse.bass_utils import run_bass_kernel_spmd
    args = {k: np.asarray(v, np.float32) for k, v in inputs.items()}
    nc = _get_nc()
    in_maps = make_in_maps(
        args["x"], args["Wq"], args["bq"], args["Wk"], args["bk"],
        args["Wv"], args["bv"], args["Wo"], args["bo"])
    res = run_bass_kernel_spmd(nc, in_maps, core_ids=list(range(8)))
    return combine_outputs(res.results, args["bo"])


# revision 21
# speedup vs baseline: 1.0370x; 1.0345x over previous
"""Multi-head causal self-attention on 8 trn2 NeuronCores, v3.

Problem: x[2,2048,1024], 16 heads x 64 dim, causal softmax attention,
QKV/O projections with biases.

Sharding: core c handles batch b=c//4, head group g=c%4 (heads 4g..4g+3).
Each core computes its 4 heads' attention plus the partial O-projection;
the host sums the 4 partials per batch and adds bo.

v3 design (vs v2):
- warm-up matmuls at kernel start: PE is idle ~4us waiting for the first
  DMAs; dummy matmuls on memset tiles fill that window AND ramp the PE
  HAM clock gate (1.2 -> 2.4GHz needs ~3.4us of sustained busy)
- projection order (k,q) x (dc0,dc1): the last proj tile is q-dc1 whose
  DVE bias-chunks don't gate attention qc0-pair0 (which needs only dc0);
  ACT k-copies split into 512-col chunks for finer overlap
- PV causal trimming: diagonal k-tiles only run PV on [lo:512] (the
  first k-tile of each accumulation is always full-width, so PSUM
  start/stop zero-region semantics stay valid); left-of-trapezoid
  memsets on p are gone
- denominator: vsb carries TWO tally columns (col 64 for even heads,
  col 65 for odd heads) so a pair's two denominator rows land on
  DIFFERENT psum partitions (64 and 65); one [2,128] selector matmul
  broadcasts both across 128 partitions, one reciprocal serves the pair
- qc3-pair1 tail: normalize per 128-col chunk, each chunk immediately
  feeding its o_stage_b matmul + add + store, instead of one monolithic
  normalize followed by 8 serialized O blocks
- xt row DMAs split across sync/gpsimd/scalar queues (serial on sync
  they gated early projection); output DMAs only on sync/gpsimd so ACT
  keeps the exp stream
"""
import os
import sys

if os.path.isdir("/opt/trn_rl_repo"):
    sys.path.insert(0, "/opt/trn_rl_repo")

import numpy as np
import ml_dtypes

import concourse.bass as bass  # noqa: F401
import concourse.tile as tile
from concourse import bacc
from concourse import mybir

F32 = mybir.dt.float32
F32R = mybir.dt.float32r
BF16 = mybir.dt.bfloat16
AF = mybir.ActivationFunctionType
ADD = mybir.AluOpType.add
MULT = mybir.AluOpType.mult

T = 2048          # sequence length
C = 1024          # model dim
HG = 4            # heads per core
HD = 64           # head dim
DG = HG * HD      # 256, projected dims per core
NF = C // 128     # 8 feature chunks
NT = T // 128     # 16 token tiles
NQ = T // 512     # 4 q-chunks
SCALE = 0.125     # 1/sqrt(64)
LAG = 8           # exp -> PV pipeline lag (in k-tiles)
NWARM = 8         # warm-up matmuls (512 cols each, ~0.4us cold)

BFNP = ml_dtypes.bfloat16


def build_kernel():
    nc = bacc.Bacc("TRN2")
    xT_d = nc.dram_tensor("xT", [C, T], BF16, kind="ExternalInput").ap()
    wq_d = nc.dram_tensor("wq", [128, NF * DG], BF16, kind="ExternalInput").ap()
    wk_d = nc.dram_tensor("wk", [128, NF * DG], BF16, kind="ExternalInput").ap()
    wv_d = nc.dram_tensor("wv", [128, NF * DG], BF16, kind="ExternalInput").ap()
    wo_d = nc.dram_tensor("wo", [2, 128, C], BF16, kind="ExternalInput").ap()
    bq_d = nc.dram_tensor("bq", [128, 2], F32, kind="ExternalInput").ap()
    bk_d = nc.dram_tensor("bk", [128, 2], F32, kind="ExternalInput").ap()
    bv_d = nc.dram_tensor("bv", [128, DG], F32, kind="ExternalInput").ap()
    out_d = nc.dram_tensor("out", [T, C], BF16, kind="ExternalOutput").ap()

    with tile.TileContext(nc) as tc:
        with tc.tile_pool(name="persist", bufs=1) as pp:
            qt = pp.tile([128, 2, T], BF16, name="qt")    # [d'128, pair, t]
            kt = pp.tile([128, 2, T], BF16, name="kt")
            # [V | even-tally | odd-tally]
            vsb = pp.tile([128, NT, HG, HD + 2], BF16, name="vsb")
            ctxT = [pp.tile([128, T], BF16, name=f"ctxT{p}") for p in range(2)]
            wo_sb = pp.tile([128, 2, C], BF16, name="wo_sb")
            bq_sb = pp.tile([128, 2], F32, name="bq_sb")
            bk_sb = pp.tile([128, 2], F32, name="bk_sb")
            bv_sb = pp.tile([128, DG], F32, name="bv_sb")
            mask2 = pp.tile([128, 2, 128], BF16, name="mask2")
            sel2 = pp.tile([128, 128], BF16, name="sel2")
            warm_w = pp.tile([128, 128], BF16, name="warm_w")
            warm_sb = pp.tile([128, 512], BF16, name="warm_sb")

            # all memsets on the DVE (idle until ~20us); gpsimd keeps its
            # queue free for DMA issues + the affine_selects (gpsimd-only
            # op). Warm-up matmuls depend only on the first two memsets.
            # exactly 0.0: in-projection warmup matmuls ADD these into
            # live psum accumulations (bit-exact no-ops that keep the PE
            # busy while the DMA-bound xt rows trickle in)
            nc.vector.memset(warm_w[:], 0.0)
            nc.vector.memset(warm_sb[:], 0.0)
            # denominator tally columns: even heads tally into psum row
            # 64, odd heads into row 65 (so the pair's denominators land
            # on distinct partitions of their two psum tiles)
            nc.vector.memset(vsb[:, :, :, HD:HD + 2], 0.0)
            for h in range(HG):
                nc.vector.memset(vsb[:, :, h, HD + (h % 2):HD + (h % 2) + 1],
                                 1.0)
            nc.vector.memset(sel2[:], 1.0)
            nc.vector.memset(mask2[:], 1.0)
            with tc.tile_pool(name="xtp", bufs=1) as xtp, \
                 tc.tile_pool(name="wp", bufs=2) as wp:
                xt = xtp.tile([128, NF, T], BF16, name="xt")

                w_srcs = {"q": wq_d, "k": wk_d, "v": wv_d}
                w_tiles = {}

                def load_w(which, eng, split=False):
                    w_tiles[which] = wp.tile([128, NF, DG], BF16,
                                             name=f"w{which}", tag="w")
                    src_v = w_srcs[which].rearrange("p (f d) -> p f d", f=NF)
                    if split:
                        # f0 alone (64KB, gates the first matmul), then the
                        # rest as one contiguous 3.5KB-per-partition transfer
                        eng.dma_start(w_tiles[which][:, 0, :], src_v[:, 0, :])
                        eng.dma_start(w_tiles[which][:, 1:NF, :],
                                      src_v[:, 1:NF, :])
                    else:
                        eng.dma_start(w_tiles[which][:], src_v)

                def load_x(eng, f):
                    eng.dma_start(xt[:, f, :], xT_d[128 * f:128 * (f + 1), :])

                # queues ordered so each transfer lands just before its
                # first consumer. xt rows load as full [128, 2048]
                # transfers only — 4KB contiguous per partition; sub-row
                # chunks (1KB lines) measured ~4x slower under 8-core HBM
                # contention.
                load_w("k", nc.scalar, split=True)   # wk f0 gates matmul #1
                load_x(nc.sync, 0)
                # wq f0 gates the q-dc0 matmuls (interleaved with k-dc0
                # from the start)
                w_tiles["q"] = wp.tile([128, NF, DG], BF16, name="wq",
                                       tag="w")
                wq_v = wq_d.rearrange("p (f d) -> p f d", f=NF)
                nc.gpsimd.dma_start(w_tiles["q"][:, 0, :], wq_v[:, 0, :])
                load_x(nc.gpsimd, 1)
                # tiny, and they gate the first kt/qt bias copies
                nc.sync.dma_start(bq_sb[:], bq_d)
                nc.sync.dma_start(bk_sb[:], bk_d)
                nc.scalar.dma_start(w_tiles["q"][:, 1:NF, :],
                                    wq_v[:, 1:NF, :])
                load_x(nc.sync, 2)
                load_x(nc.scalar, 3)
                load_x(nc.sync, 4)
                load_x(nc.scalar, 5)
                load_x(nc.sync, 6)
                load_x(nc.gpsimd, 7)
                load_w("v", nc.gpsimd)
                nc.sync.dma_start(bv_sb[:], bv_d)
                for p in range(2):
                    nc.gpsimd.dma_start(wo_sb[:, p, :], wo_d[p])

                # gpsimd-only affine_selects, after its DMA issues (none
                # of these are needed before ~55us)
                # selector for the denominator broadcast matmul: psum row
                # 64 -> out partitions 0..63, row 65 -> partitions
                # 64..127. (partition starts must be 32-aligned, so rows
                # 64:66 are carved with two affine_selects)
                # keep iff col - 128p + 64 >= 0 (p rel. to partition 64)
                nc.gpsimd.affine_select(
                    out=sel2[64:66, :], in_=sel2[64:66, :],
                    compare_op=mybir.AluOpType.is_ge, fill=0.0,
                    base=64, pattern=[[1, 128]], channel_multiplier=-128)
                # keep iff -col + 63 + 128p >= 0
                nc.gpsimd.affine_select(
                    out=sel2[64:66, :], in_=sel2[64:66, :],
                    compare_op=mybir.AluOpType.is_ge, fill=0.0,
                    base=63, pattern=[[-1, 128]], channel_multiplier=128)
                # multiplicative triangle mask: 1 where col >= partition
                for j in range(2):
                    nc.gpsimd.affine_select(
                        out=mask2[:, j, :],
                        in_=mask2[:, j, :],
                        compare_op=mybir.AluOpType.is_ge,
                        fill=0.0,
                        base=0,
                        pattern=[[1, 128]],
                        channel_multiplier=-1,
                    )

                # ---- PE warm-up: fill the DMA wait, ramp the clock ----
                with tc.tile_pool(name="wrm", bufs=1, space="PSUM") as wmp:
                    warm_ps = wmp.tile([128, 512], F32, name="warm_ps")
                    for _ in range(NWARM):
                        nc.tensor.matmul(warm_ps[:], warm_w[:],
                                         warm_sb[:], start=True, stop=True)

                # ---- QK projections: psum [128, 2048] per (dst, dc) ----
                _sid_p, _ = nc.enter_named_scope("proj", False)
                with tc.tile_pool(name="pjp", bufs=2, space="PSUM") as pjp:
                    def copy_chunk(wkey, dst, ps, dc, b_sb, t4):
                        sl = slice(512 * t4, 512 * (t4 + 1))
                        if wkey == "k":
                            # ACT is idle during proj: bias + copy
                            nc.scalar.activation(
                                dst[:, dc, sl], ps[:, sl], AF.Identity,
                                bias=b_sb[:, dc:dc + 1])
                        else:
                            nc.vector.tensor_scalar_add(
                                dst[:, dc, sl], ps[:, sl], b_sb[:, dc:dc + 1])

                    # dc0: k and q interleaved f-outer. The start is
                    # aggregate-DMA-bound (~2us per 512KB xt row); a
                    # single proj tile eats a row per ~1us and stalls, two
                    # tiles together match the delivery rate so the PE
                    # stays continuously busy (HAM stays warm).
                    combos = (("k", kt, bk_sb), ("q", qt, bq_sb))
                    ps0 = {wkey: pjp.tile([128, T], F32, name=f"pj{wkey}0",
                                          tag="pj")
                           for wkey, _, _ in combos}
                    for f in range(NF - 1):
                        for wkey, dst, b_sb in combos:
                            for t4 in range(NQ):
                                nc.tensor.matmul(
                                    ps0[wkey][:, 512 * t4:512 * (t4 + 1)],
                                    w_tiles[wkey][:, f, 0:128],
                                    xt[:, f, 512 * t4:512 * (t4 + 1)],
                                    start=(f == 0), stop=False,
                                )
                        if f < NF - 2:
                            # xt rows arrive every ~3.5us (8 cores share
                            # HBM at start) but a round only takes ~2us:
                            # pad with +=0 matmuls so the PE never idles
                            # and the HAM clock stays warm
                            for w_i in range(3):
                                nc.tensor.matmul(
                                    ps0[combos[w_i % 2][0]][
                                        :, 512 * (w_i % NQ):
                                        512 * (w_i % NQ + 1)],
                                    warm_w[:],
                                    warm_sb[:],
                                    start=False, stop=False,
                                )
                    # last f-round chunk-by-chunk with inline copies so
                    # the psum banks retire while the tail chunks stream
                    for t4 in range(NQ):
                        for wkey, dst, b_sb in combos:
                            nc.tensor.matmul(
                                ps0[wkey][:, 512 * t4:512 * (t4 + 1)],
                                w_tiles[wkey][:, NF - 1, 0:128],
                                xt[:, NF - 1, 512 * t4:512 * (t4 + 1)],
                                start=False, stop=True,
                            )
                        for wkey, dst, b_sb in combos:
                            copy_chunk(wkey, dst, ps0[wkey], 0, b_sb, t4)
                    # dc1: x fully resident by now; t4-outer so each
                    # 512-chunk's psum retires right away (the attention
                    # pools reuse these banks)
                    ps1 = {wkey: pjp.tile([128, T], F32, name=f"pj{wkey}1",
                                          tag="pj")
                           for wkey, _, _ in combos}
                    for t4 in range(NQ):
                        for wkey, dst, b_sb in combos:
                            for f in range(NF):
                                nc.tensor.matmul(
                                    ps1[wkey][:, 512 * t4:512 * (t4 + 1)],
                                    w_tiles[wkey][:, f, 128:256],
                                    xt[:, f, 512 * t4:512 * (t4 + 1)],
                                    start=(f == 0), stop=(f == NF - 1),
                                )
                        for wkey, dst, b_sb in combos:
                            copy_chunk(wkey, dst, ps1[wkey], 1, b_sb, t4)
                nc.leave_named_scope("proj", _sid_p, False)

                # ---- phase B + V-proj/O-proj as PE filler ----
                wv_sb = w_tiles["v"]
                with tc.tile_pool(name="pp2", bufs=12) as pbuf, \
                     tc.tile_pool(name="opp", bufs=8) as opp, \
                     tc.tile_pool(name="outp", bufs=6) as outp, \
                     tc.tile_pool(name="dnp", bufs=4) as dnp, \
                     tc.tile_pool(name="sps", bufs=2, space="PSUM") as sps, \
                     tc.tile_pool(name="cps", bufs=2, space="PSUM") as cps, \
                     tc.tile_pool(name="vop", bufs=2, space="PSUM") as vop:

                    filler = []

                    def v_group(t):
                        def emit():
                            ps = vop.tile([128, DG], F32, name="vps", tag="vo")
                            for f in range(NF):
                                nc.tensor.matmul(
                                    ps[:],
                                    xt[:, f, 128 * t:128 * (t + 1)],
                                    wv_sb[:, f, :],
                                    start=(f == 0), stop=(f == NF - 1),
                                )
                            nc.vector.tensor_tensor(
                                vsb[:, t, :, 0:HD],
                                ps[:].rearrange("p (h d) -> p h d", h=HG),
                                bv_sb[:].rearrange("p (h d) -> p h d", h=HG),
                                ADD)
                        return emit

                    def o_stage_a(t0, c2, store):
                        def emit():
                            o_ps = vop.tile([128, 512], F32, name="opsA",
                                            tag="vo")
                            nc.tensor.matmul(
                                o_ps[:], ctxT[0][:, t0:t0 + 128],
                                wo_sb[:, 0, 512 * c2:512 * (c2 + 1)],
                                start=True, stop=True)
                            part = opp.tile([128, 512], BF16, name="opart",
                                            tag="op")
                            nc.vector.tensor_copy(part[:], o_ps[:])
                            store[(t0, c2)] = part
                        return emit

                    def o_stage_b(t0, c2, store):
                        o_ps = vop.tile([128, 512], F32, name="opsB",
                                        tag="vo")
                        nc.tensor.matmul(
                            o_ps[:], ctxT[1][:, t0:t0 + 128],
                            wo_sb[:, 1, 512 * c2:512 * (c2 + 1)],
                            start=True, stop=True)
                        o_sb = outp.tile([128, 512], BF16, name="osbB",
                                         tag="osb")
                        nc.vector.tensor_tensor(
                            o_sb[:], o_ps[:], store[(t0, c2)], ADD)
                        (nc.sync, nc.gpsimd)[c2].dma_start(
                            out_d[t0:t0 + 128, 512 * c2:512 * (c2 + 1)],
                            o_sb[:])

                    def o_group(t0, c2, alt=[0]):
                        def emit():
                            o_ps = vop.tile([128, 512], F32, name="ops",
                                            tag="vo")
                            for p in range(2):
                                nc.tensor.matmul(
                                    o_ps[:],
                                    ctxT[p][:, t0:t0 + 128],
                                    wo_sb[:, p, 512 * c2:512 * (c2 + 1)],
                                    start=(p == 0), stop=(p == 1),
                                )
                            o_sb = outp.tile([128, 512], BF16, name="osb",
                                             tag="osb")
                            nc.vector.tensor_copy(o_sb[:], o_ps[:])
                            nc.sync.dma_start(
                                out_d[t0:t0 + 128, 512 * c2:512 * (c2 + 1)],
                                o_sb[:])
                        return emit

                    for t in range(NT):
                        filler.append(v_group(t))

                    def pop_filler(n=1):
                        for _ in range(n):
                            if filler:
                                filler.pop(0)()

                    o_parts = {}

                    def make_norm(qc, pair, cps_t, granular):
                        """Denominator broadcast + normalize for a finished
                        pair. Deferred to just after the NEXT pair's first
                        S batch so the PE never idles on the pd casts."""
                        heads = (2 * pair, 2 * pair + 1)

                        def emit():
                            h0, h1 = heads
                            pd = dnp.tile([66, 512], BF16, name="pd",
                                          tag="dnb")
                            # odd head first as a 2-row aligned copy (its
                            # row 64 is a zero tally, overwritten next by
                            # the even head's single aligned row). At the
                            # tail the casts go to the (by then idle) ACT
                            # so the DVE chain stays short.
                            ceng = nc.scalar if granular else nc.vector
                            if granular:
                                ceng.activation(pd[64:66, :],
                                                cps_t[h1][64:66, :], AF.Copy)
                                ceng.activation(pd[64:65, :],
                                                cps_t[h0][64:65, :], AF.Copy)
                            else:
                                ceng.tensor_copy(pd[64:66, :],
                                                 cps_t[h1][64:66, :])
                                ceng.tensor_copy(pd[64:65, :],
                                                 cps_t[h0][64:65, :])
                            bc_ps = vop.tile([128, 512], F32, name="bc",
                                             tag="vo")
                            nc.tensor.matmul(
                                bc_ps[:],
                                sel2[64:66, :],
                                pd[64:66, :],
                                start=True, stop=True)
                            bcr = dnp.tile([128, 512], F32, name="bcr",
                                           tag="bcr")
                            nc.vector.reciprocal_approx_fast(
                                out=bcr[:], in_=bc_ps[:])
                            if not granular:
                                for j, h in enumerate(heads):
                                    nc.vector.tensor_tensor(
                                        ctxT[pair][64 * j:64 * j + 64,
                                                   512 * qc:512 * (qc + 1)],
                                        cps_t[h][0:HD, :],
                                        bcr[64 * j:64 * j + 64, :], MULT)
                            else:
                                # final chunk: normalize per 128-col piece,
                                # each immediately feeding its O block
                                for tt in range(4):
                                    csl = slice(128 * tt, 128 * (tt + 1))
                                    for j, h in enumerate(heads):
                                        nc.vector.tensor_tensor(
                                            ctxT[1][64 * j:64 * j + 64,
                                                    512 * qc + 128 * tt:
                                                    512 * qc + 128 * (tt + 1)],
                                            cps_t[h][0:HD, csl],
                                            bcr[64 * j:64 * j + 64, csl],
                                            MULT)
                                    for c2 in range(2):
                                        o_stage_b(512 * qc + 128 * tt, c2,
                                                  o_parts)
                        return emit

                    # (stamp, emit_fn): flushed one global step after they
                    # were appended, right behind that step's S batch
                    pending_norm = []
                    gstep = [0]

                    for qc in range(NQ):
                        _sid_a, _ = nc.enter_named_scope(f"attn{qc}", False)
                        nkt = 4 * qc + 4
                        # both pairs in one pipelined stream: the next
                        # pair's S batches run during the previous pair's
                        # PV drain, so the ACT exp stream never restarts
                        # cold and the PE has S work at every boundary
                        batches = [(p, kb) for p in (0, 1)
                                   for kb in range(0, nkt, 2)]
                        n_b = len(batches)
                        cps_all = {}
                        pts = {}
                        los = {}
                        for step in range(n_b + LAG // 2):
                            g = gstep[0]
                            if step < n_b:
                                pair, kb = batches[step]
                                if kb == 0:
                                    cps_all[pair] = {
                                        h: cps.tile([66, 512], F32,
                                                    name=f"cps{h}",
                                                    tag="cps")
                                        for h in (2 * pair, 2 * pair + 1)}
                                sts = {}
                                for ki in (kb, kb + 1):
                                    r = ki - 4 * qc
                                    lo = 128 * r if r > 0 else 0
                                    sts[ki] = (lo, sps.tile(
                                        [128, 2, 512], F32,
                                        name="s_ps", tag="s"))
                                    for j in range(2):
                                        nc.tensor.matmul(
                                            sts[ki][1][:, j, lo:512],
                                            kt[64 * j:64 * j + 64, pair,
                                               128 * ki:128 * (ki + 1)],
                                            qt[64 * j:64 * j + 64, pair,
                                               512 * qc + lo:
                                               512 * (qc + 1)],
                                            start=True, stop=True)
                                for ki in (kb, kb + 1):
                                    lo, s_ps = sts[ki]
                                    r = ki - 4 * qc
                                    p_t = pbuf.tile([128, 2, 512], BF16,
                                                    name="p", tag="p")
                                    nc.scalar.activation(
                                        p_t[:, :, lo:512],
                                        s_ps[:, :, lo:512],
                                        AF.Exp, scale=SCALE)
                                    if r >= 0:
                                        # zero the above-diag triangle
                                        # (p is SBUF bf16 -> Pool-able)
                                        nc.gpsimd.tensor_tensor(
                                            p_t[:, :, lo:lo + 128],
                                            p_t[:, :, lo:lo + 128],
                                            mask2[:], MULT)
                                    pts[(pair, ki)] = p_t
                                    los[(pair, ki)] = lo
                            # deferred normalizes land here: behind the S
                            # batch just emitted, ahead of this step's PV
                            # (whose psum banks they must release first)
                            while pending_norm and g > pending_norm[0][0]:
                                pending_norm.pop(0)[1]()
                            if step == 2:
                                pop_filler(2)
                            elif step > 2:
                                pop_filler()
                            bi = step - LAG // 2
                            if bi >= 0:
                                pair2, kb2 = batches[bi]
                                heads2 = (2 * pair2, 2 * pair2 + 1)
                                for k in (kb2, kb2 + 1):
                                    pk = pts.pop((pair2, k))
                                    lo = los.pop((pair2, k))
                                    for j, h in enumerate(heads2):
                                        nc.tensor.matmul(
                                            cps_all[pair2][h][:, lo:512],
                                            vsb[:, k, h, :],
                                            pk[:, j, lo:512],
                                            start=(k == 0),
                                            stop=(k == nkt - 1),
                                        )
                                if kb2 == nkt - 2:
                                    last = (qc == NQ - 1 and pair2 == 1)
                                    cps_p = cps_all[pair2]
                                    if qc == NQ - 1 and pair2 == 0:
                                        norm_fn = make_norm(qc, pair2, cps_p,
                                                            granular=False)

                                        def entry(norm_fn=norm_fn, qc=qc):
                                            norm_fn()
                                            # stage_a reads the ctxT the
                                            # norm just wrote — append only
                                            # now so no pop precedes it
                                            for tt in range(4):
                                                for c2 in range(2):
                                                    filler.append(o_stage_a(
                                                        512 * qc + 128 * tt,
                                                        c2, o_parts))
                                    else:
                                        entry = make_norm(qc, pair2, cps_p,
                                                          granular=last)
                                    if last:
                                        entry()
                                    else:
                                        pending_norm.append((g, entry))
                            gstep[0] += 1
                        nc.leave_named_scope(f"attn{qc}", _sid_a, False)
                        if qc < NQ - 1:
                            for tt in range(4):
                                for c2 in range(2):
                                    filler.append(o_group(
                                        512 * qc + 128 * tt, c2))
                    while pending_norm:
                        pending_norm.pop(0)[1]()
                    while filler:
                        pop_filler()

    nc.compile()
    return nc


_NC_CACHE = None


def _get_nc():
    global _NC_CACHE
    if _NC_CACHE is None:
        _NC_CACHE = build_kernel()
    return _NC_CACHE


def make_in_maps(x, Wq, bq, Wk, bk, Wv, bv, Wo, bo):
    in_maps = []
    for c in range(8):
        b, g = c // 4, c % 4
        sl = slice(256 * g, 256 * (g + 1))
        bqg = np.ascontiguousarray(bq[sl].reshape(2, 128).T)
        bkg = np.ascontiguousarray(bk[sl].reshape(2, 128).T)
        bvg = np.ascontiguousarray(np.tile(bv[sl][None, :], (128, 1)))
        in_maps.append({
            "xT": np.ascontiguousarray(x[b].T).astype(BFNP),
            "wq": np.ascontiguousarray(
                Wq[:, sl].reshape(NF, 128, DG).transpose(1, 0, 2)
                .reshape(128, NF * DG)).astype(BFNP),
            "wk": np.ascontiguousarray(
                Wk[:, sl].reshape(NF, 128, DG).transpose(1, 0, 2)
                .reshape(128, NF * DG)).astype(BFNP),
            "wv": np.ascontiguousarray(
                Wv[:, sl].reshape(NF, 128, DG).transpose(1, 0, 2)
                .reshape(128, NF * DG)).astype(BFNP),
            "wo": np.ascontiguousarray(Wo[sl, :].reshape(2, 128, C)).astype(BFNP),
            "bq": bqg.astype(np.float32),
            "bk": bkg.astype(np.float32),
            "bv": bvg.astype(np.float32),
        })
    return in_maps


def combine_outputs(results, bo):
    out = np.empty((2, T, C), np.float32)
    for b in range(2):
        acc = results[4 * b]["out"].astype(np.float32).copy()
        for g in range(1, 4):
            acc += results[4 * b + g]["out"]
        out[b] = acc + bo[None, :]
    return out


def kernel(**inputs):
    from concourse.bass_utils import run_bass_kernel_spmd
    args = {k: np.asarray(v, np.float32) for k, v in inputs.items()}
    nc = _get_nc()
    in_maps = make_in_maps(
        args["x"], args["Wq"], args["bq"], args["Wk"], args["bk"],
        args["Wv"], args["bv"], args["Wo"], args["bo"])
    res = run_bass_kernel_spmd(nc, in_maps, core_ids=list(range(8)))
    return combine_outputs(res.results, args["bo"])
